# revision 73
# baseline (speedup 1.0000x reference)
# GQA attention block on 8 Trainium2 NeuronCores — fully-pipelined bf16.
# Sharding: core = (batch b in {0,1}) x (tensor-parallel t in {0..3}).
# Each core: batch row b, 4 query heads {4t..4t+3}, 2 kv heads {2t, 2t+1}.
# W_Q/W_K/W_V split column-wise (per-head), W_O row-wise; the 4 TP partial
# outputs per batch are summed on the host (the "all-reduce").
#
# Schedule (measured ~95% PE occupancy, ~312us/core):
#  - phase 1: K0/K1/Q0 projection units each drag 4 V columns chunk-major
#    through the DMA feed window (the feed, not the PE, paces the start);
#    Q1/Q2/Q3's units are WOVEN into attention heads (0,0)-(0,3) as PE
#    fill, so the exp stream starts ~40us before the projections finish
#  - attention head = 8 pairs of t-chunks: scores (PE) -> exp (ACT,
#    N=1024 per instruction) -> ctx accumulate (PE); scores prefetch runs
#    one pair ahead and CROSSES head boundaries, so ACT never idles
#  - output projection trails attention by one sc block, 2 matmuls woven
#    per pair; its PSUM->SBUF copies split 2 DVE / 2 ACT so consecutive
#    bank releases never convoy on one engine queue
#  - softmax denominator: DVE running adds over ex tiles (two bf16 lanes,
#    one op per pair) + one gpsimd partition_all_reduce per head; the
#    reciprocal+normalize are DEFERRED into the next head and pinned
#    behind its g3 add (add_dep_helper) — otherwise the strict-FIFO DVE
#    queue blocks ~3.5us on the gpsimd wait once per head
#  - final head's denominator chain is split into s-halves to shorten the
#    kernel tail; one outproj group is drained before its recip chain
#  - RoPE in bf16 (2x DVE modes), output written as bf16
import math
import sys

sys.path.insert(0, "/opt/trn_rl_repo")

import ml_dtypes
import numpy as np

import concourse.bacc as bacc
import concourse.bass as bass
import concourse.bass_isa as bass_isa
import concourse.mybir as mybir
import concourse.tile as tile
from contextlib import ExitStack

BF = mybir.dt.bfloat16
F32 = mybir.dt.float32
bfnp = ml_dtypes.bfloat16

EMB = 2048
HEADS = 16
G = 2
HD = 128          # head dim
KV = HEADS // G   # 8 kv heads
B = 2
S = 2048
NCORES = 8
TP = 4
HQ = HEADS // TP       # 4 q heads per core
HKV = KV // TP         # 2 kv heads per core
NE = EMB // 128        # 16 contraction chunks
SC4 = S // 512         # 4 s-chunks of 512
SC16 = S // 128        # 16 s-chunks of 128
SCALE = 1.0 / math.sqrt(float(EMB))

_NC = None


def _build_program(loop_n=None):
    nc = bacc.Bacc("TRN2", target_bir_lowering=False, debug=False)

    xT = nc.dram_tensor("xT", (EMB, S), BF, kind="ExternalInput")
    wq = nc.dram_tensor("wq", (EMB, HQ * HD), BF, kind="ExternalInput")
    wk = nc.dram_tensor("wk", (EMB, HKV * HD), BF, kind="ExternalInput")
    wv = nc.dram_tensor("wv", (EMB, HKV * HD), BF, kind="ExternalInput")
    wo = nc.dram_tensor("wo", (HQ * HD, EMB), BF, kind="ExternalInput")
    cosT = nc.dram_tensor("cosT", (HD, S), BF, kind="ExternalInput")
    sinT = nc.dram_tensor("sinT", (HD, S), BF, kind="ExternalInput")
    out = nc.dram_tensor("out", (S, EMB), BF, kind="ExternalOutput")

    with tile.TileContext(nc) as tc, ExitStack() as ctx:
        persist = ctx.enter_context(tc.tile_pool(name="persist", bufs=1))
        # roped Q (jb 0..3) and K (jb 4..5), bf16: [d, jb, sc, s512]
        qk_sb = persist.tile([128, HQ + HKV, SC4, 512], BF)
        # V in [t, d] layout: [t_part, t_chunk, kvl*128+d]
        v_sb = persist.tile([128, SC16, HKV * HD], BF)
        ctx_sb = persist.tile([128, HQ, SC4, 512], BF)   # [d, head, sc, s]
        wo_sb = persist.tile([128, HQ, SC4, 512], BF)    # [d, head, ec, e]
        xt_sb = persist.tile([128, NE, S], BF)
        wqs = persist.tile([128, NE, HQ * HD], BF)
        wks = persist.tile([128, NE, HKV * HD], BF)
        wvs = persist.tile([128, NE, HKV * HD], BF)
        cos_sb = persist.tile([128, SC4, 512], BF)
        sin_sb = persist.tile([128, SC4, 512], BF)

        # batched input loads: few multi-dim DMAs (the SP sequencer pays
        # ~0.6us dispatch per DMA). xT is split so its completion semaphores
        # fire progressively and the first projection can start early; wk/wv
        # chunks are interleaved with it because the first unit's V matmuls
        # consume wv chunk c together with xt chunk c.
        # The first unit consumes xt, wk AND wv chunk-by-chunk, so all three
        # stream interleaved in consumption order, fine-grained (2-chunk
        # granules) so completion semaphores fire progressively.  wq/cos/
        # sin/wo follow -- they are consumed much later.
        xTr = xT.rearrange("(c p) s -> p c s", p=128)
        wkr = wk.rearrange("(c p) j -> p c j", p=128)
        wvr = wv.rearrange("(c p) j -> p c j", p=128)
        # xt streams in s-halves: pass A of every unit (and V columns 0-7)
        # only reads s 0:1024, so the DMA-paced opening window waits for
        # half the bytes; the second half lands while pass A computes.
        for ci in range(8):
            cs = slice(2 * ci, 2 * ci + 2)
            nc.sync.dma_start(out=xt_sb[:, cs, 0:1024], in_=xTr[:, cs, 0:1024])
            # K0's unit reads only wk columns 0:128; K1's half follows the
            # xt stream so it doesn't steal feed bandwidth in the first
            # ~20us (unit K0 is DMA-paced there)
            nc.sync.dma_start(out=wks[:, cs, 0:128], in_=wkr[:, cs, 0:128])
            nc.sync.dma_start(out=wvs[:, cs, :], in_=wvr[:, cs, :])
        nc.sync.dma_start(out=wks[:, 0:8, 128:256], in_=wkr[:, 0:8, 128:256])
        nc.sync.dma_start(out=wks[:, 8:16, 128:256], in_=wkr[:, 8:16, 128:256])
        for ci in range(8):
            cs = slice(2 * ci, 2 * ci + 2)
            nc.sync.dma_start(
                out=xt_sb[:, cs, 1024:2048], in_=xTr[:, cs, 1024:2048]
            )
        nc.sync.dma_start(out=wqs, in_=wq.rearrange("(c p) j -> p c j", p=128))
        nc.sync.dma_start(out=cos_sb, in_=cosT.rearrange("p (sc s) -> p sc s", s=512))
        nc.sync.dma_start(out=sin_sb, in_=sinT.rearrange("p (sc s) -> p sc s", s=512))
        nc.sync.dma_start(
            out=wo_sb, in_=wo.rearrange("(jb p) (ec e) -> p jb ec e", p=128, e=512)
        )

        # PSUM budget (8 banks): pairs 2x2 + accp 2 + oacc 2
        pairs = ctx.enter_context(tc.tile_pool(name="pairs", bufs=2, space="PSUM"))
        accp = ctx.enter_context(tc.tile_pool(name="accp", bufs=2, space="PSUM"))
        oacc = ctx.enter_context(tc.tile_pool(name="oacc", bufs=2, space="PSUM"))
        # expool slots are shared with the phase-1 rope tiles (same shape,
        # disjoint lifetime) via a single tag
        expool = ctx.enter_context(tc.tile_pool(name="expool", bufs=6))
        dccp = ctx.enter_context(tc.tile_pool(name="dccp", bufs=2))
        darp = ctx.enter_context(tc.tile_pool(name="darp", bufs=2))
        rbp = ctx.enter_context(tc.tile_pool(name="rbp", bufs=1))
        outs = ctx.enter_context(tc.tile_pool(name="outs", bufs=3))

        warm = persist.tile([128, 256], BF)

        def _phases():
            # Pre-warm the ACT "exp" table set while the input DMAs stream:
            # otherwise the first real exp pays the ~2.7us table load in the
            # middle of the kernel.
            nc.vector.memset(warm, 0.0)
            nc.scalar.activation(
                warm[:, 0:16], warm[:, 0:16], mybir.ActivationFunctionType.Exp
            )
            # PE warm-up: dummy matmuls on zeros while the first input DMAs
            # land, so the HAM activity window starts ramping the PE clock
            # before the real projection stream begins (output never read).
            wps = oacc.tile([128, 512], F32, tag="oacc", name="wps")
            for _ in range(14):
                nc.tensor.matmul(
                    wps[:, 0:256], warm[:, 0:128], warm, start=True, stop=True
                )

            # ---------------- Phase 1: projections + RoPE ----------------
            def rope(jb, scp, pt):
                xs = expool.tile([128, 2, 512], BF, tag="ex")
                if jb in (2, 3):
                    # last Q units: keep the ACT queue clear so attention's
                    # first exp isn't stuck behind these copies
                    nc.vector.tensor_copy(xs, pt)
                else:
                    nc.scalar.copy(xs, pt)
                xw = expool.tile([128, 2, 512], BF, tag="ex")
                nc.sync.dma_start(out=xw[0:64, :, :], in_=xs[64:128, :, :])
                nc.sync.dma_start(out=xw[64:128, :, :], in_=xs[0:64, :, :])
                csl = slice(2 * scp, 2 * scp + 2)
                nc.vector.tensor_mul(xs, xs, cos_sb[:, csl, :])
                nc.vector.tensor_mul(xw, xw, sin_sb[:, csl, :])
                nc.vector.tensor_add(qk_sb[:, jb, csl, :], xs, xw)

            def jsl_of(jb):
                if jb < HQ:
                    return wqs, slice(jb * 128, (jb + 1) * 128)
                kvl = jb - HQ
                return wks, slice(kvl * 128, (kvl + 1) * 128)

            def qk_passA(jb, vsts):
                # pass A of a projection unit: V columns (t-chunks < 8, i.e.
                # xt s-range 0:1024) + the unit's first s-half (pt0),
                # chunk-major against the streaming first-half xt feed.
                # Consuming only half of xt here halves the bytes the
                # DMA-paced opening window has to wait for.
                w_sb, jsl = jsl_of(jb)
                pt0 = pairs.tile([128, 2, 512], F32, tag="pairs", name=f"ptA_{jb}")
                pvs = []
                for i, st in enumerate(vsts):
                    pool = accp if i < 2 else oacc
                    pvs.append(
                        pool.tile([128, 512], F32,
                                  tag="accp" if i < 2 else "oacc",
                                  name=f"pv_{jb}_{st}")
                    )
                # V matmuls lead the qk matmuls by LEAD chunks: at unit
                # boundaries the first qk matmul waits for the previous
                # unit's rope copies to release the scores psum slots, and
                # the leading V matmuls (own psum pool) fill that latency
                LEAD = 5
                for c in range(NE + LEAD):
                    if c < NE:
                        for i, st in enumerate(vsts):
                            nc.tensor.matmul(
                                pvs[i][:, 0:HKV * HD],
                                xt_sb[:, c, st * 128:(st + 1) * 128],
                                wvs[:, c, :],
                                start=(c == 0), stop=(c == NE - 1),
                            )
                    if c >= LEAD:
                        cq = c - LEAD
                        lhsT = w_sb[:, cq, jsl]
                        for k in range(2):
                            nc.tensor.matmul(
                                pt0[:, k, :], lhsT,
                                xt_sb[:, cq, k * 512:(k + 1) * 512],
                                start=(cq == 0), stop=(cq == NE - 1),
                            )
                rope(jb, 0, pt0)
                for i, st in enumerate(vsts):
                    nc.scalar.copy(v_sb[:, st, :], pvs[i][:, 0:HKV * HD])

            def qk_passB(jb):
                # pass B: the unit's second s-half (pt1) against the
                # second-half xt, which streamed in while pass A computed
                w_sb, jsl = jsl_of(jb)
                pt1 = pairs.tile([128, 2, 512], F32, tag="pairs", name=f"ptB_{jb}")
                for c in range(NE):
                    lhsT = w_sb[:, c, jsl]
                    for k in range(2):
                        sck = 2 + k
                        nc.tensor.matmul(
                            pt1[:, k, :], lhsT,
                            xt_sb[:, c, sck * 512:(sck + 1) * 512],
                            start=(c == 0), stop=(c == NE - 1),
                        )
                rope(jb, 1, pt1)

            def do_v(sts):
                for st in sts:
                    pv = accp.tile([128, 512], F32, tag="accp")
                    for c in range(NE):
                        nc.tensor.matmul(
                            pv[:, 0:HKV * HD],
                            xt_sb[:, c, st * 128:(st + 1) * 128],
                            wvs[:, c, :],
                            start=(c == 0), stop=(c == NE - 1),
                        )
                    nc.scalar.copy(v_sb[:, st, :], pv[:, 0:HKV * HD])

            # K first (attention h=0 needs it), each early unit dragging 4 V
            # columns chunk-major through the DMA feed window (the feed is
            # the limit there, so the extra V matmuls soak DMA-wait slack).
            # All pass A's (first-half-xt consumers) run before any pass B
            # (second-half consumers) so the opening window only waits for
            # half of xt.  V columns 8-15 read second-half xt and run last.
            # Q1/Q2/Q3's units are NOT emitted here: they are woven into
            # attention heads (0,0)/(0,1)/(0,2) as PE fill work (see
            # qk_stream below), which starts the exp stream ~25us earlier;
            # each head h only needs Q_h, which the weave of head h-1
            # finished.
            qk_passA(HQ, [0, 1, 2, 3])
            qk_passA(HQ + 1, [4, 5, 6, 7])
            qk_passA(0, [])
            qk_passB(HQ)
            qk_passB(HQ + 1)
            qk_passB(0)
            do_v([8, 9, 10, 11])
            do_v([12, 13, 14, 15])

            # single-s-chunk variant of rope, for the woven Q units (their
            # psum accumulator is a single [128,512] oacc bank)
            def rope1(jb, sck, pt1):
                xs = expool.tile([128, 512], BF, tag="ex")
                nc.scalar.copy(xs, pt1)
                xw = expool.tile([128, 512], BF, tag="ex")
                nc.sync.dma_start(out=xw[0:64, :], in_=xs[64:128, :])
                nc.sync.dma_start(out=xw[64:128, :], in_=xs[0:64, :])
                nc.vector.tensor_mul(xs, xs, cos_sb[:, sck, :])
                nc.vector.tensor_mul(xw, xw, sin_sb[:, sck, :])
                nc.vector.tensor_add(qk_sb[:, jb, sck, :], xs, xw)

            def qk_stream(jb):
                w_sb, jsl = jsl_of(jb)
                for sck in range(4):
                    pt1 = oacc.tile([128, 512], F32, tag="oacc")
                    for c in range(NE):
                        nc.tensor.matmul(
                            pt1, w_sb[:, c, jsl],
                            xt_sb[:, c, sck * 512:(sck + 1) * 512],
                            start=(c == 0), stop=(c == NE - 1),
                        )
                        if c == NE - 1:
                            rope1(jb, sck, pt1)
                        yield

            # ---------- Phase 2+3: attention + output projection ----------
            # Interleaved at head granularity: outproj(sc-1) group so4=j is
            # emitted after attention head (sc, j).  By then the denominator
            # chain (gpsimd all-reduce + recip + mul) for ALL of sc-1's heads
            # has drained, so the outproj matmuls never block the PE queue,
            # and they serve as fill work for the exp-gated attention stream.
            # All PSUM->SBUF copies run on DVE: the ACT engine carries only
            # the exp stream, which paces attention.
            def scores_for(sc_, h_, g):
                kvjb_ = HQ + h_ // 2
                sp = pairs.tile([128, 2, 512], F32, tag="pairs")
                for k in range(2):
                    tcn = 2 * g + k
                    nc.tensor.matmul(
                        sp[:, k, :],
                        qk_sb[:, kvjb_, tcn // 4, (tcn % 4) * 128:(tcn % 4) * 128 + 128],
                        qk_sb[:, h_, sc_, :],
                        start=True, stop=True,
                    )
                return sp

            def attn_head(sc, h, filler, prev_tail, sp0, nxt, weave_n=2,
                          weave_from_g=0):
                kvl = h // 2
                cps = accp.tile([128, 512], F32, tag="accp")
                dacc = dccp.tile([128, 2, 512], BF, tag="dacc")

                # scores run one pair ahead of exp/ctx so the static PE
                # stream never blocks on the activation latency; the
                # prefetch crosses head boundaries (sp0 came from the
                # previous head, and this head emits the next head's first
                # pair at g=7) so ACT never idles at a boundary
                sp_next = sp0 if sp0 is not None else scores_for(sc, h, 0)
                sp0_next = None
                ex0 = None
                for g in range(8):        # pairs of 128-wide t-chunks
                    sp = sp_next
                    if g < 7:
                        sp_next = scores_for(sc, h, g + 1)
                    elif nxt is not None:
                        sp0_next = scores_for(nxt[0], nxt[1], 0)
                    ex = expool.tile([128, 2, 512], BF, tag="ex")
                    nc.scalar.activation(
                        ex, sp, mybir.ActivationFunctionType.Exp, scale=SCALE,
                    )
                    # weave_n filler matmuls woven into each pair BEFORE the
                    # ctx matmuls: PE fill work with no ACT dependency, and
                    # placing it here gives exp(g) an extra ~450ns before
                    # ctx(g) reaches the PE queue head (fewer isolated-MM
                    # restarts).  Emitted before the dacc add too, so the
                    # filler's psum-releasing copies never wait out the
                    # add's exp pacing in the strict-FIFO DVE queue.
                    if g >= weave_from_g:
                        for _ in range(weave_n):
                            next(filler, None)
                    for k in range(2):
                        nc.tensor.matmul(
                            cps,
                            v_sb[:, 2 * g + k, kvl * 128:(kvl + 1) * 128],
                            ex[:, k, :],
                            start=(g == 0 and k == 0), stop=(g == 7 and k == 1),
                        )
                    # two running denominator lanes -> one DVE op per pair;
                    # the first add consumes the g=0 and g=1 tiles together
                    if g == 0:
                        ex0 = ex
                    elif g == 1:
                        nc.vector.tensor_add(dacc, ex0, ex)
                    else:
                        av = nc.vector.tensor_add(dacc, dacc, ex)
                        if g == 3:
                            add3 = av.ins
                    # the previous head's recip+mul are emitted here (g=6),
                    # ANCHORED behind this head's g3 add: the recip waits on
                    # a 3.5us gpsimd reduce, and without the anchor the
                    # scheduler hoists it right behind the fold where the
                    # wait blocks the whole strict-FIFO DVE queue (ex-tile
                    # recycling, outproj-psum-releasing casts)
                    if g == 6 and prev_tail is not None:
                        prev_tail(add3)

                # fold + partition reduce start now (no engine-blocking
                # waits); the reduce runs during the next head's g0-g2
                if nxt is not None:
                    nc.vector.tensor_add(
                        dacc[:, 0, :], dacc[:, 0, :], dacc[:, 1, :]
                    )
                    dar = darp.tile([128, 512], F32, tag="dar")
                    nc.gpsimd.partition_all_reduce(
                        dar, dacc[:, 0, :], 128, bass_isa.ReduceOp.add
                    )

                    def tail(anchor=None):
                        rb = rbp.tile([128, 512], F32, tag="rb")
                        rc = nc.vector.reciprocal_approx_fast(rb, dar)
                        if anchor is not None:
                            bass._add_dep_helper(
                                rc.ins, anchor, sync=False,
                                reason="recip ordered behind next head's g3 "
                                       "add (gpsimd reduce surely done)",
                            )
                        nc.vector.tensor_mul(ctx_sb[:, h, sc, :], cps, rb)
                else:
                    # final head: the whole tail chain is on the kernel's
                    # critical path (the last outproj groups wait on it).
                    # Split it into s-halves so the first half's gpsimd
                    # reduce+recip+mul finish ~1.8us earlier and the first
                    # tail outproj groups (so4=0,1) start sooner.
                    halves = []
                    for hf in range(2):
                        ssl = slice(256 * hf, 256 * hf + 256)
                        nc.vector.tensor_add(
                            dacc[:, 0, ssl], dacc[:, 0, ssl], dacc[:, 1, ssl]
                        )
                        dar = darp.tile([128, 256], F32, tag="dar")
                        nc.gpsimd.partition_all_reduce(
                            dar, dacc[:, 0, ssl], 128, bass_isa.ReduceOp.add
                        )
                        halves.append((ssl, dar))

                    def tail(anchor=None):
                        for hf, (ssl, dar) in enumerate(halves):
                            rb = rbp.tile([128, 256], F32, tag="rb")
                            rc = nc.vector.reciprocal_approx_fast(rb, dar)
                            if anchor is not None and hf == 0:
                                bass._add_dep_helper(
                                    rc.ins, anchor, sync=False,
                                    reason="final recips behind the drained "
                                           "group's DVE cast",
                                )
                            nc.vector.tensor_mul(
                                ctx_sb[:, h, sc, ssl], cps[:, ssl], rb
                            )
                return tail, sp0_next

            # one outproj row-block (128 out rows x full EMB) of sc,
            # as a generator yielding after each matmul
            def outproj_stream(sc, so4):
                tail_grp = sc == SC4 - 1 and so4 == 3
                ot4 = outs.tile([128, SC4, 512], BF, tag="ot")
                so = sc * 4 + so4
                for ec in range(SC4):
                    ops = oacc.tile([128, 512], F32, tag="oacc")
                    for hl in range(HQ):
                        ret = nc.tensor.matmul(
                            ops,
                            ctx_sb[:, hl, sc, so4 * 128:(so4 + 1) * 128],
                            wo_sb[:, hl, ec, :],
                            start=(hl == 0), stop=(hl == HQ - 1),
                        )
                        if hl == HQ - 1:
                            # copy split 2 DVE / 2 ACT: alternating engines
                            # decouples consecutive psum-bank releases, so a
                            # lagging DVE FIFO can't stall the ec+2 matmuls
                            if ec % 2 == 1:
                                ret = nc.scalar.copy(ot4[:, ec, :], ops)
                            else:
                                ret = nc.vector.tensor_copy(ot4[:, ec, :], ops)
                            if tail_grp:
                                # final row-block: per-chunk DMAs so the
                                # kernel's last semaphore rides a
                                # quarter-size transfer
                                nc.sync.dma_start(
                                    out=out[so * 128:(so + 1) * 128,
                                            ec * 512:(ec + 1) * 512],
                                    in_=ot4[:, ec, :],
                                )
                            elif ec == SC4 - 1:
                                nc.sync.dma_start(
                                    out=out[so * 128:(so + 1) * 128, :].rearrange(
                                        "p (ec e) -> p ec e", e=512
                                    ),
                                    in_=ot4,
                                )
                        yield ret

            def chain_streams(items):
                for sc, so4 in items:
                    yield from outproj_stream(sc, so4)

            def _chain_iters(*its):
                for it in its:
                    yield from it

            # outproj trails attention by one sc block plus one head-step
            # (so group (sc,0)'s hl=3 matmul is always emitted after the
            # deferred mul that writes ctx_sb[sc, h3])
            groups = [(sc, so4) for sc in range(SC4) for so4 in range(4)]
            filler = chain_streams(groups)
            empty = iter(())
            seq = [(sc, h) for sc in range(SC4) for h in range(HQ)]
            prev_tail = None
            sp0 = None
            for si, (sc, h) in enumerate(seq):
                nxt = seq[si + 1] if si + 1 < len(seq) else None
                if si == 0:
                    qk_fill = _chain_iters(
                        qk_stream(1), qk_stream(2), qk_stream(3)
                    )
                if si < 4:
                    fil, wn = qk_fill, 7
                elif si == 4:
                    fil, wn = empty, 2
                else:
                    fil, wn = filler, 2
                prev_tail, sp0 = attn_head(
                    sc, h, fil, prev_tail, sp0, nxt, weave_n=wn
                )
            # drain one outproj group BEFORE the final head's recip chain:
            # its gpsimd reduces run under that group's matmuls, and the
            # recips land in the DVE FIFO after the group's psum-releasing
            # casts instead of blocking them
            drain = [next(filler, None) for _ in range(16)]
            # anchor the final head's recips behind the drained group's ec2
            # DVE cast (yield index 11): without this the scheduler hoists
            # them ahead of the casts, where their gpsimd-reduce waits block
            # the strict-FIFO DVE queue and stall the tail outproj matmuls
            anc = drain[11].ins if drain[11] is not None else None
            prev_tail(anc)
            for _ in filler:
                pass

        if loop_n is not None:
            with tc.For_i(0, loop_n, 1):
                _phases()
        else:
            _phases()

    nc.compile()
    return nc


def _get_nc():
    global _NC
    if _NC is None:
        _NC = _build_program()
    return _NC


def _rope_tables():
    half = HD // 2
    inv_freq = 1.0 / (10000.0 ** (np.arange(half, dtype=np.float64) * 2.0 / HD))
    ang = np.arange(S, dtype=np.float64)[:, None] * inv_freq[None, :]  # (S, 64)
    cos = np.concatenate([np.cos(ang), np.cos(ang)], axis=1).T  # (128, S)
    sin = np.concatenate([-np.sin(ang), np.sin(ang)], axis=1).T  # pre-signed
    return (np.ascontiguousarray(cos).astype(bfnp),
            np.ascontiguousarray(sin).astype(bfnp))


def build_in_maps(x, W_Q, W_K, W_V, W_O):
    x = np.asarray(x, dtype=np.float32)
    W_Q = np.asarray(W_Q, dtype=np.float32)
    W_K = np.asarray(W_K, dtype=np.float32)
    W_V = np.asarray(W_V, dtype=np.float32)
    W_O = np.asarray(W_O, dtype=np.float32)
    cos, sin = _rope_tables()
    in_maps = []
    xTb = [np.ascontiguousarray(x[b].T).astype(bfnp) for b in range(B)]
    for b in range(B):
        for t in range(TP):
            qheads = list(range(HQ * t, HQ * t + HQ))
            kvheads = [HKV * t + i for i in range(HKV)]
            idxq = [d * HEADS + h for h in qheads for d in range(HD)]
            idxkv = [d * KV + kv for kv in kvheads for d in range(HD)]
            rows_o = [h * HD + d for h in qheads for d in range(HD)]
            in_maps.append(dict(
                xT=xTb[b],
                wq=np.ascontiguousarray(W_Q[idxq, :].T).astype(bfnp),
                wk=np.ascontiguousarray(W_K[idxkv, :].T).astype(bfnp),
                wv=np.ascontiguousarray(W_V[idxkv, :].T).astype(bfnp),
                wo=np.ascontiguousarray(W_O[:, rows_o].T).astype(bfnp),
                cosT=cos,
                sinT=sin,
            ))
    return in_maps


def emulate_core(m):
    """Numpy emulation of the device math for one core's in_map."""
    xT = np.asarray(m["xT"], np.float32)      # (E, S)
    wq = np.asarray(m["wq"], np.float32)      # (E, 512)
    wk = np.asarray(m["wk"], np.float32)
    wv = np.asarray(m["wv"], np.float32)
    wo = np.asarray(m["wo"], np.float32)      # (512, E)
    cos = np.asarray(m["cosT"], np.float32)   # (128, S)
    sin = np.asarray(m["sinT"], np.float32)

    def bfq(a):
        return a.astype(bfnp).astype(np.float32)

    qT = bfq(wq.T @ xT)                       # (512, S)
    kT = bfq(wk.T @ xT)
    vT = bfq(wv.T @ xT)

    def rope(blkT):  # (128, S)
        xw = np.concatenate([blkT[64:], blkT[:64]], axis=0)
        return bfq(blkT * cos + xw * sin)

    ctxs = []
    for h in range(HQ):
        qh = rope(qT[h * 128:(h + 1) * 128])
        kvl = h // 2
        kh = rope(kT[kvl * 128:(kvl + 1) * 128])
        vh = vT[kvl * 128:(kvl + 1) * 128]
        scoresT = kh.T @ qh * SCALE           # (t, s)
        w = bfq(np.exp(scoresT))
        den = w.sum(axis=0)
        ctxT = bfq((vh @ w) / den[None, :])
        ctxs.append(ctxT)
    ctx = np.concatenate(ctxs, axis=0)        # (512, S)
    return bfq(ctx.T @ wo)


def combine_outs(outs):
    out = np.empty((B, S, EMB), dtype=np.float32)
    for b in range(B):
        acc = np.asarray(outs[TP * b]).astype(np.float32)
        for t in range(1, TP):
            acc = acc + np.asarray(outs[TP * b + t]).astype(np.float32)
        out[b] = acc
    return out


LAST_RESULTS = None


def kernel(x, W_Q, W_K, W_V, W_O):
    global LAST_RESULTS
    from concourse.bass_utils import run_bass_kernel_spmd

    nc = _get_nc()
    in_maps = build_in_maps(x, W_Q, W_K, W_V, W_O)
    res = run_bass_kernel_spmd(nc, in_maps, list(range(NCORES)))
    LAST_RESULTS = res
    outs = [r["out"] for r in res.results]
    return combine_outs(outs)

